# revision 43
# baseline (speedup 1.0000x reference)
"""FCOS decode + class-specific NMS on 8 Trainium2 NeuronCores (Bass/Tile).

Problem: N=8192 candidate boxes, 80 classes.
  reference = (decoded boxes,
               greedy-NMS keep mask (original order),
               score-descending argsort permutation)

Algorithm / sharding
--------------------
Class-specific NMS offsets boxes by class_id*(max_coord+1), which makes
cross-class IoU exactly 0, so greedy NMS in global score order is exactly
per-class greedy NMS (each class's members kept in score order).  We:

  * host: argsort scores (the `order` output + within-class ordering), group
    boxes per class into a padded (80, 136) slot layout, rotate the class
    axis per core so one SPMD program serves all 8 cores (core k owns global
    classes 10k..10k+9 which sit at local slots 0..9 of its rotated input).
  * device (per core): decode all padded boxes (f32, exact same op sequence
    as the reference), global max coordinate, class offsets, then pairwise
    suppression bits S[c,k,j] = (3*inter > areaA+areaB) == (IoU > 0.5) for
    its 10 own classes, batched as (128 suppressor rows, 10 classes x 136
    candidate cols) ops on the full 128-lane vector engine; S is collapsed
    per class into a (10, 128*136) sweep tile (classes on partitions), then
    a 128-step greedy suppression sweep runs all 10 classes in parallel
    (1 scalar_tensor_tensor op per step).  Each core also decodes a
    1024-row shard of the boxes output.
  * host: scatter the padded keep mask back to original indices.

The multiply-form predicate (inter > (areaA+areaB)/3) differs from the
reference's divide-and-compare only within ~1ulp of the IoU==0.5 boundary;
the data's minimum decision margin is 5.9e-4, so decisions match exactly.
"""

import sys

for _p in ("/opt/trn_rl_repo", "/root/.axon_site/_ro/trn_rl_repo"):
    if _p not in sys.path:
        sys.path.append(_p)

import numpy as np

import concourse.bacc as bacc
import concourse.bass as bass
import concourse.bass_isa as bass_isa
import concourse.tile as tile
from concourse import mybir
from concourse.bass_utils import run_bass_kernel_spmd


def _install_ntff_shim():
    """Some images lack antenv.axon_hooks; bass_utils imports it when tracing
    is requested (BASS_TRACE).  Provide the same ctypes-based NTFF hook so a
    traced run works, or degrade to no-trace instead of crashing."""
    import contextlib
    import ctypes
    import types

    try:
        import antenv.axon_hooks  # noqa: F401

        return
    except ImportError:
        pass

    hook = None
    try:
        lib = ctypes.CDLL("/opt/axon/libaxon_pjrt.so")
        if hasattr(lib, "axon_start_nrt_profile"):
            lib.axon_start_nrt_profile.argtypes = [
                ctypes.POINTER(ctypes.c_int64),
                ctypes.c_size_t,
            ]
            lib.axon_start_nrt_profile.restype = ctypes.c_int64
            lib.axon_stop_nrt_profile.argtypes = [ctypes.c_char_p]
            lib.axon_stop_nrt_profile.restype = ctypes.c_int64

            @contextlib.contextmanager
            def hook(output_dir, device_ids):
                import jax

                jax.devices()
                if device_ids:
                    ids = (ctypes.c_int64 * len(device_ids))(*device_ids)
                    rc = lib.axon_start_nrt_profile(ids, len(device_ids))
                else:
                    rc = lib.axon_start_nrt_profile(None, 0)
                if rc != 0:
                    raise RuntimeError(f"axon_start_nrt_profile rc={rc}")
                try:
                    yield
                finally:
                    lib.axon_stop_nrt_profile(str(output_dir).encode())
    except OSError:
        pass

    mod = types.ModuleType("antenv.axon_hooks")
    mod.get_axon_ntff_profile_hook = lambda h=hook: h
    mod.set_axon_ntff_profile_hook = lambda h: None
    sys.modules["antenv.axon_hooks"] = mod


_install_ntff_shim()

F32 = mybir.dt.float32
OP = mybir.AluOpType

N = 8192
NCLS = 80
W = 136            # padded per-class width (max class size 129); 80*136 = 10880
FLAT = NCLS * W    # 10880 = 128 * 85
PF = FLAT // 128   # 85
NCORES = 8
CPC = NCLS // NCORES   # classes per core
CW = CPC * W           # 1360 slots owned per core
NSH = N // NCORES      # boxes-output rows per core

LAST_RUN = None  # BassKernelResults of the most recent device run (for test.py)

_PROGRAM_CACHE = {}


def _bcast_j(ap, nj):
    """Append a step-0 (broadcast) inner dim of size nj to an AP."""
    return bass.AP(tensor=ap.tensor, offset=ap.offset, ap=[*ap.ap, [0, nj]])


def _build_program(smax: int, stride: float):
    """One SPMD Bass program, identical for all 8 cores."""
    nc = bacc.Bacc(
        "TRN2",
        target_bir_lowering=False,
        debug=False,
        enable_asserts=True,
        num_devices=NCORES,
    )

    pd = nc.dram_tensor("pd", (4, FLAT), F32, kind="ExternalInput")
    pl = nc.dram_tensor("pl", (2, FLAT), F32, kind="ExternalInput")
    pcls = nc.dram_tensor("pcls", (FLAT,), F32, kind="ExternalInput")
    shd = nc.dram_tensor("shd", (NSH, 4), F32, kind="ExternalInput")
    shl = nc.dram_tensor("shl", (NSH, 2), F32, kind="ExternalInput")

    boxes_out = nc.dram_tensor("boxes_out", (NSH, 4), F32, kind="ExternalOutput")
    keep_out = nc.dram_tensor("keep_out", (CPC, W), F32, kind="ExternalOutput")

    nsweep = min(smax - 1, 128)  # suppressor rows that can reach a real box

    with tile.TileContext(nc) as tc:
        with (
            tc.tile_pool(name="work", bufs=1) as work,
            tc.tile_pool(name="dram", bufs=1, space="DRAM") as dpool,
        ):
            # ---- stage 1: decode all padded boxes, offsets, write coord planes
            td = work.tile([128, 4, PF], F32)
            nc.scalar.dma_start(out=td, in_=pd.ap().rearrange("q (p f) -> p q f", p=128))
            tl = work.tile([128, 2, PF], F32)
            nc.sync.dma_start(out=tl, in_=pl.ap().rearrange("q (p f) -> p q f", p=128))
            tcls = work.tile([128, PF], F32)
            nc.gpsimd.dma_start(out=tcls, in_=pcls.ap().rearrange("(p f) -> p f", p=128))

            dr = work.tile([128, 4, PF], F32)
            nc.vector.tensor_scalar_max(dr, td, 0.0)  # clip(deltas, 0)

            x1 = work.tile([128, PF], F32)
            y1 = work.tile([128, PF], F32)
            x2 = work.tile([128, PF], F32)
            y2 = work.tile([128, PF], F32)
            # x1 = lx - s*d0 computed as (d0*-s) + lx  (bit-identical: a-b == a+(-b))
            nc.vector.scalar_tensor_tensor(x1, dr[:, 0], -stride, tl[:, 0], OP.mult, OP.add)
            nc.vector.scalar_tensor_tensor(y1, dr[:, 1], -stride, tl[:, 1], OP.mult, OP.add)
            nc.vector.scalar_tensor_tensor(x2, dr[:, 2], stride, tl[:, 0], OP.mult, OP.add)
            nc.vector.scalar_tensor_tensor(y2, dr[:, 3], stride, tl[:, 1], OP.mult, OP.add)

            # global max coordinate: x2>=x1, y2>=y1 always (deltas clipped >=0),
            # so max over x2,y2 suffices; pad slots decode to 0 <= real max.
            mx = work.tile([128, PF], F32)
            nc.vector.tensor_tensor(mx, x2, y2, OP.max)
            red = work.tile([128, 1], F32)
            nc.vector.tensor_reduce(red, mx, axis=mybir.AxisListType.X, op=OP.max)
            m0b = work.tile([128, 1], F32)
            nc.gpsimd.partition_all_reduce(m0b, red, 128, bass_isa.ReduceOp.max)
            m1b = work.tile([128, 1], F32)
            nc.vector.tensor_scalar_add(m1b, m0b, 1.0)  # max_coordinate + 1

            off = work.tile([128, PF], F32)
            nc.vector.tensor_scalar_mul(off, tcls, m1b)  # class_id * (maxc+1)

            # offset coords + areas (kept in SBUF; only the core's first CW
            # slots = its own 10 classes are consumed downstream)
            offc = []
            for qi, t in enumerate((x1, y1, x2, y2)):
                to = work.tile([128, PF], F32, tag=f"coord_o{qi}")
                nc.vector.tensor_tensor(to, t, off, OP.add)
                offc.append(to)
            x1o, y1o, x2o, y2o = offc
            wdt = work.tile([128, PF], F32)
            nc.vector.tensor_tensor(wdt, x2o, x1o, OP.subtract)
            hgt = work.tile([128, PF], F32)
            nc.vector.tensor_tensor(hgt, y2o, y1o, OP.subtract)
            area = work.tile([128, PF], F32)
            nc.vector.tensor_tensor(area, wdt, hgt, OP.mult)

            # ---- stage 2: boxes-output shard decode (1024 original-order rows)
            sd = work.tile([128, NSH // 128, 4], F32)
            nc.scalar.dma_start(out=sd, in_=shd.ap().rearrange("(p a) c -> p a c", p=128))
            sl = work.tile([128, NSH // 128, 2], F32)
            nc.sync.dma_start(out=sl, in_=shl.ap().rearrange("(p a) c -> p a c", p=128))
            sdr = work.tile([128, NSH // 128, 4], F32)
            nc.vector.tensor_scalar_max(sdr, sd, 0.0)
            bx = work.tile([128, NSH // 128, 4], F32)
            nc.vector.scalar_tensor_tensor(bx[:, :, 0], sdr[:, :, 0], -stride, sl[:, :, 0], OP.mult, OP.add)
            nc.vector.scalar_tensor_tensor(bx[:, :, 1], sdr[:, :, 1], -stride, sl[:, :, 1], OP.mult, OP.add)
            nc.vector.scalar_tensor_tensor(bx[:, :, 2], sdr[:, :, 2], stride, sl[:, :, 0], OP.mult, OP.add)
            nc.vector.scalar_tensor_tensor(bx[:, :, 3], sdr[:, :, 3], stride, sl[:, :, 1], OP.mult, OP.add)
            nc.scalar.dma_start(
                out=boxes_out.ap().rearrange("(p a) c -> p a c", p=128), in_=bx
            )

            # ---- stage 3: batched suppression bits for the core's 10 classes
            # The core's own 10 classes occupy slots 0..CW-1 = partitions 0..15
            # of the (128, 85) coord tiles (CW = 16*85).  Per coord plane:
            # extract those slots to one partition (column operand, then
            # partition-broadcast to 128 lanes) and to DRAM (bounce for the
            # free->partition row gather).
            planes = (x1o, y1o, x2o, y2o, area)
            NPP = CW // PF  # 16 partitions holding the core's classes
            colsd = dpool.tile([5, CW], F32)
            colB = [
                work.tile([128, CW], F32, tag=f"colB{qi}", name=f"colB{qi}")
                for qi in range(5)
            ]
            rows = work.tile([128, 5, CPC], F32)
            eng_a = (nc.scalar, nc.sync)
            bc_eng = (None, None, nc.sync, nc.scalar, nc.sync)
            for qi, pt in enumerate(planes):
                eng_a[(qi + 1) % 2].dma_start(
                    out=colsd[qi].rearrange("(b f) -> b f", b=NPP), in_=pt[0:NPP, :]
                )
                if qi < 2:
                    # earliest-needed planes: on-chip broadcast (gpsimd),
                    # running parallel to the queue-limited broadcast-DMAs
                    cols_q = work.tile([1, CW], F32, tag=f"cols{qi}")
                    eng_a[qi % 2].dma_start(
                        out=cols_q[0:1, :].rearrange("a (b f) -> a b f", b=NPP),
                        in_=pt[0:NPP, :],
                    )
                    nc.gpsimd.partition_broadcast(colB[qi], cols_q)
                else:
                    # later planes: broadcast-DMA from the DRAM copy — no
                    # gpsimd write-drain stall for the first vector reader
                    bc_eng[qi].dma_start(
                        out=colB[qi],
                        in_=bass.AP(tensor=colsd.tensor,
                                    offset=colsd.offset + qi * CW,
                                    ap=[[0, 128], [1, CW]]),
                    )
                nc.sync.dma_start(
                    out=rows[:, qi, :],
                    in_=bass.AP(tensor=colsd.tensor,
                                offset=colsd.offset + qi * CW,
                                ap=[[1, 128], [W, CPC]]),
                )

            def rowv(qi):  # (128, CPC, W) j-broadcast view of plane qi's rows
                return _bcast_j(rows[:, qi, :], W)

            # TT max/min run at half DVE rate; tensor_scalar max/min are full
            # rate but the row operand varies per class, so do the four
            # row-dependent ops per class (scalar = this class's row coord)
            # and fuse min+subtract into one scalar_tensor_tensor.
            # plane-major order: the first reader of each freshly-broadcast
            # colB plane stalls on the gpsimd drain, so pay that stall once
            # per plane (not once per class-chain link)
            sh3 = [128, CPC, W]
            ltx = work.tile(sh3, F32)
            lty = work.tile(sh3, F32)
            wq = work.tile(sh3, F32)
            hq = work.tile(sh3, F32)
            for c in range(CPC):
                nc.vector.tensor_scalar_max(
                    ltx[:, c, :], colB[0][:, c * W : (c + 1) * W], rows[:, 0, c : c + 1]
                )
            for c in range(CPC):
                nc.vector.tensor_scalar_max(
                    lty[:, c, :], colB[1][:, c * W : (c + 1) * W], rows[:, 1, c : c + 1]
                )
            for c in range(CPC):
                nc.vector.scalar_tensor_tensor(
                    wq[:, c, :], colB[2][:, c * W : (c + 1) * W], rows[:, 2, c : c + 1],
                    ltx[:, c, :], OP.min, OP.subtract,
                )
            for c in range(CPC):
                nc.vector.scalar_tensor_tensor(
                    hq[:, c, :], colB[3][:, c * W : (c + 1) * W], rows[:, 3, c : c + 1],
                    lty[:, c, :], OP.min, OP.subtract,
                )
            hc = work.tile(sh3, F32)
            nc.scalar.activation(hc, hq, mybir.ActivationFunctionType.Relu)
            inter = work.tile(sh3, F32)
            nc.vector.scalar_tensor_tensor(inter, wq, 0.0, hc, OP.max, OP.mult)
            asum = work.tile(sh3, F32)
            nc.vector.tensor_tensor(
                asum, colB[4].rearrange("p (c j) -> p c j", c=CPC), rowv(4), OP.add
            )

            # suppression bits in bf16 (0/1 exact): halves the collapse DMA
            # bytes; is_gt split per class so each class's collapse starts
            # as soon as its bits exist instead of after the whole batch
            BF16 = mybir.dt.bfloat16
            sbits = work.tile(sh3, BF16)
            keep_a = work.tile([CPC, W], BF16)
            keep_b = work.tile([CPC, W], BF16)
            nc.vector.memset(keep_a, 1.0)
            nc.vector.memset(keep_b, 1.0)
            s3 = work.tile(sh3, F32)
            nc.scalar.mul(s3, asum, 1.0 / 3.0)
            nc.vector.tensor_tensor(sbits, inter, s3, OP.is_gt)
            # partition flip via DRAM bounce: contiguous (k, c, j) write, then
            # strided read back as (c-partition, k-major).  Split in row
            # quarters so sweep steps 0-31 only wait for the first quarter
            # and the rest transfers underneath the running sweep.
            KQ = 8
            sshalf = [
                work.tile([CPC, KQ * W], BF16, tag=f"ssweep{h}", name=f"ssweep{h}")
                for h in range(128 // KQ)
            ]
            for h in range(128 // KQ):
                sb_d = dpool.tile([KQ * CW], BF16, tag=f"sb_d{h}")
                nc.scalar.dma_start(
                    out=sb_d.rearrange("(k f) -> k f", k=KQ),
                    in_=sbits[KQ * h : KQ * (h + 1)],
                )
                rd_eng = nc.sync if h % 2 == 0 else nc.scalar
                rd_eng.dma_start(
                    out=sshalf[h].rearrange("c (k j) -> c k j", k=KQ),
                    in_=bass.AP(tensor=sb_d.tensor, offset=sb_d.offset,
                                ap=[[W, CPC], [CW, KQ], [1, W]]),
                )

            # ---- stage 4: greedy suppression sweep, all classes in parallel.
            # Ping-pong buffers instead of in-place so each step is a clean
            # cross-tile RAW; keep[] only ever decreases, so column j's final
            # value (written at step j-1) is the elementwise min of the two.
            bufs = (keep_a, keep_b)
            for k in range(nsweep):
                src, dst = bufs[k % 2], bufs[(k + 1) % 2]
                stile = sshalf[k // KQ]
                kk = k % KQ
                # dst[j] = (S[k,j]*src[k]) < src[j]   for j in (k, smax)
                nc.vector.scalar_tensor_tensor(
                    dst[:, k + 1 : smax],
                    stile[:, kk * W + k + 1 : kk * W + smax],
                    src[:, k : k + 1],
                    src[:, k + 1 : smax],
                    OP.mult,
                    OP.is_lt,
                )

            keepm = work.tile([CPC, W], BF16)
            nc.vector.tensor_tensor(keepm, keep_a, keep_b, OP.min)
            keepf = work.tile([CPC, W], F32)
            nc.vector.tensor_copy(keepf, keepm)
            nc.sync.dma_start(out=keep_out.ap(), in_=keepf)

    nc.compile()
    return nc


def kernel(deltas, locations, scores, class_ids, stride):
    global LAST_RUN
    deltas = np.ascontiguousarray(np.asarray(deltas, dtype=np.float32))
    locations = np.ascontiguousarray(np.asarray(locations, dtype=np.float32))
    scores = np.asarray(scores, dtype=np.float32)
    class_ids = np.asarray(class_ids, dtype=np.int32)
    s = float(stride)
    assert deltas.shape == (N, 4) and locations.shape == (N, 2)

    # ---- host: ordering + class grouping (layout only; all math on device)
    order = np.argsort(-scores, kind="stable").astype(np.int32)
    rank = np.empty(N, np.int64)
    rank[order] = np.arange(N)
    sort_key = np.lexsort((rank, class_ids))          # class-major, score-rank minor
    counts = np.bincount(class_ids, minlength=NCLS)
    smax = int(counts.max())
    assert smax <= 129, f"class size {smax} exceeds supported 129"

    starts = np.zeros(NCLS, np.int64)
    starts[1:] = np.cumsum(counts)[:-1]
    sorted_cls = class_ids[sort_key].astype(np.int64)
    within = np.arange(N) - np.repeat(starts, counts)
    slots = sorted_cls * W + within                    # padded slot per box

    pd_c = np.zeros((4, NCLS, W), np.float32)
    pl_c = np.zeros((2, NCLS, W), np.float32)
    pcls_c = np.zeros((NCLS, W), np.float32)
    pd_c.reshape(4, FLAT)[:, slots] = deltas[sort_key].T
    pl_c.reshape(2, FLAT)[:, slots] = locations[sort_key].T
    pcls_c.reshape(FLAT)[slots] = sorted_cls.astype(np.float32)

    key = (smax, s)
    if key not in _PROGRAM_CACHE:
        _PROGRAM_CACHE[key] = _build_program(smax, s)
    nc = _PROGRAM_CACHE[key]

    in_maps = []
    for k in range(NCORES):
        r = -(k * CPC)
        in_maps.append(
            {
                "pd": np.ascontiguousarray(np.roll(pd_c, r, axis=1).reshape(4, FLAT)),
                "pl": np.ascontiguousarray(np.roll(pl_c, r, axis=1).reshape(2, FLAT)),
                "pcls": np.ascontiguousarray(np.roll(pcls_c, r, axis=0).reshape(FLAT)),
                "shd": deltas[k * NSH : (k + 1) * NSH],
                "shl": locations[k * NSH : (k + 1) * NSH],
            }
        )

    res = run_bass_kernel_spmd(nc, in_maps, list(range(NCORES)))
    LAST_RUN = res
    results = res.results

    boxes = np.concatenate([results[k]["boxes_out"] for k in range(NCORES)], axis=0)
    keep_pad = np.concatenate(
        [results[k]["keep_out"].reshape(CPC, W) for k in range(NCORES)], axis=0
    )
    keep_orig = np.empty(N, bool)
    keep_orig[sort_key] = keep_pad.reshape(FLAT)[slots] > 0.5
    return boxes, keep_orig, order


# revision 46
# speedup vs baseline: 1.0127x; 1.0127x over previous
"""FCOS decode + class-specific NMS on 8 Trainium2 NeuronCores (Bass/Tile).

Problem: N=8192 candidate boxes, 80 classes.
  reference = (decoded boxes,
               greedy-NMS keep mask (original order),
               score-descending argsort permutation)

Algorithm / sharding
--------------------
Class-specific NMS offsets boxes by class_id*(max_coord+1), which makes
cross-class IoU exactly 0, so greedy NMS in global score order is exactly
per-class greedy NMS (each class's members kept in score order).  We:

  * host: argsort scores (the `order` output + within-class ordering), group
    boxes per class into a padded (80, 136) slot layout, rotate the class
    axis per core so one SPMD program serves all 8 cores (core k owns global
    classes 10k..10k+9 which sit at local slots 0..9 of its rotated input).
  * device (per core): decode all padded boxes (f32, exact same op sequence
    as the reference), global max coordinate, class offsets, then pairwise
    suppression bits S[c,k,j] = (3*inter > areaA+areaB) == (IoU > 0.5) for
    its 10 own classes, batched as (128 suppressor rows, 10 classes x 136
    candidate cols) ops on the full 128-lane vector engine; S is collapsed
    per class into a (10, 128*136) sweep tile (classes on partitions), then
    a 128-step greedy suppression sweep runs all 10 classes in parallel
    (1 scalar_tensor_tensor op per step).  Each core also decodes a
    1024-row shard of the boxes output.
  * host: scatter the padded keep mask back to original indices.

The multiply-form predicate (inter > (areaA+areaB)/3) differs from the
reference's divide-and-compare only within ~1ulp of the IoU==0.5 boundary;
the data's minimum decision margin is 5.9e-4, so decisions match exactly.
"""

import sys

for _p in ("/opt/trn_rl_repo", "/root/.axon_site/_ro/trn_rl_repo"):
    if _p not in sys.path:
        sys.path.append(_p)

import numpy as np

import concourse.bacc as bacc
import concourse.bass as bass
import concourse.bass_isa as bass_isa
import concourse.tile as tile
from concourse import mybir
from concourse.bass_utils import run_bass_kernel_spmd


def _install_ntff_shim():
    """Some images lack antenv.axon_hooks; bass_utils imports it when tracing
    is requested (BASS_TRACE).  Provide the same ctypes-based NTFF hook so a
    traced run works, or degrade to no-trace instead of crashing."""
    import contextlib
    import ctypes
    import types

    try:
        import antenv.axon_hooks  # noqa: F401

        return
    except ImportError:
        pass

    hook = None
    try:
        lib = ctypes.CDLL("/opt/axon/libaxon_pjrt.so")
        if hasattr(lib, "axon_start_nrt_profile"):
            lib.axon_start_nrt_profile.argtypes = [
                ctypes.POINTER(ctypes.c_int64),
                ctypes.c_size_t,
            ]
            lib.axon_start_nrt_profile.restype = ctypes.c_int64
            lib.axon_stop_nrt_profile.argtypes = [ctypes.c_char_p]
            lib.axon_stop_nrt_profile.restype = ctypes.c_int64

            @contextlib.contextmanager
            def hook(output_dir, device_ids):
                import jax

                jax.devices()
                if device_ids:
                    ids = (ctypes.c_int64 * len(device_ids))(*device_ids)
                    rc = lib.axon_start_nrt_profile(ids, len(device_ids))
                else:
                    rc = lib.axon_start_nrt_profile(None, 0)
                if rc != 0:
                    raise RuntimeError(f"axon_start_nrt_profile rc={rc}")
                try:
                    yield
                finally:
                    lib.axon_stop_nrt_profile(str(output_dir).encode())
    except OSError:
        pass

    mod = types.ModuleType("antenv.axon_hooks")
    mod.get_axon_ntff_profile_hook = lambda h=hook: h
    mod.set_axon_ntff_profile_hook = lambda h: None
    sys.modules["antenv.axon_hooks"] = mod


_install_ntff_shim()

F32 = mybir.dt.float32
OP = mybir.AluOpType

N = 8192
NCLS = 80
W = 136            # padded per-class width (max class size 129); 80*136 = 10880
FLAT = NCLS * W    # 10880 = 128 * 85
PF = FLAT // 128   # 85
NCORES = 8
CPC = NCLS // NCORES   # classes per core
CW = CPC * W           # 1360 slots owned per core
NSH = N // NCORES      # boxes-output rows per core

LAST_RUN = None  # BassKernelResults of the most recent device run (for test.py)

_PROGRAM_CACHE = {}


def _bcast_j(ap, nj):
    """Append a step-0 (broadcast) inner dim of size nj to an AP."""
    return bass.AP(tensor=ap.tensor, offset=ap.offset, ap=[*ap.ap, [0, nj]])


def _build_program(smax: int, stride: float):
    """One SPMD Bass program, identical for all 8 cores."""
    nc = bacc.Bacc(
        "TRN2",
        target_bir_lowering=False,
        debug=False,
        enable_asserts=True,
        num_devices=NCORES,
    )

    pd = nc.dram_tensor("pd", (4, FLAT), F32, kind="ExternalInput")
    pl = nc.dram_tensor("pl", (2, FLAT), F32, kind="ExternalInput")
    pcls = nc.dram_tensor("pcls", (FLAT,), F32, kind="ExternalInput")
    shd = nc.dram_tensor("shd", (NSH, 4), F32, kind="ExternalInput")
    shl = nc.dram_tensor("shl", (NSH, 2), F32, kind="ExternalInput")

    boxes_out = nc.dram_tensor("boxes_out", (NSH, 4), F32, kind="ExternalOutput")
    keep_out = nc.dram_tensor("keep_out", (CPC, W), F32, kind="ExternalOutput")

    nsweep = min(smax - 1, 128)  # suppressor rows that can reach a real box

    with tile.TileContext(nc) as tc:
        with (
            tc.tile_pool(name="work", bufs=1) as work,
            tc.tile_pool(name="dram", bufs=1, space="DRAM") as dpool,
        ):
            # ---- stage 1: decode all padded boxes, offsets, write coord planes
            td = work.tile([128, 4, PF], F32)
            nc.scalar.dma_start(out=td, in_=pd.ap().rearrange("q (p f) -> p q f", p=128))
            tl = work.tile([128, 2, PF], F32)
            nc.sync.dma_start(out=tl, in_=pl.ap().rearrange("q (p f) -> p q f", p=128))
            tcls = work.tile([128, PF], F32)
            nc.gpsimd.dma_start(out=tcls, in_=pcls.ap().rearrange("(p f) -> p f", p=128))

            dr = work.tile([128, 4, PF], F32)
            nc.vector.tensor_scalar_max(dr, td, 0.0)  # clip(deltas, 0)

            x1 = work.tile([128, PF], F32)
            y1 = work.tile([128, PF], F32)
            x2 = work.tile([128, PF], F32)
            y2 = work.tile([128, PF], F32)
            # x1 = lx - s*d0 computed as (d0*-s) + lx  (bit-identical: a-b == a+(-b))
            nc.vector.scalar_tensor_tensor(x1, dr[:, 0], -stride, tl[:, 0], OP.mult, OP.add)
            nc.vector.scalar_tensor_tensor(y1, dr[:, 1], -stride, tl[:, 1], OP.mult, OP.add)
            nc.vector.scalar_tensor_tensor(x2, dr[:, 2], stride, tl[:, 0], OP.mult, OP.add)
            nc.vector.scalar_tensor_tensor(y2, dr[:, 3], stride, tl[:, 1], OP.mult, OP.add)

            # global max coordinate: x2>=x1, y2>=y1 always (deltas clipped >=0),
            # so max over x2,y2 suffices; pad slots decode to 0 <= real max.
            mx = work.tile([128, PF], F32)
            nc.vector.tensor_tensor(mx, x2, y2, OP.max)
            red = work.tile([128, 1], F32)
            nc.vector.tensor_reduce(red, mx, axis=mybir.AxisListType.X, op=OP.max)
            m0b = work.tile([128, 1], F32)
            nc.gpsimd.partition_all_reduce(m0b, red, 128, bass_isa.ReduceOp.max)
            m1b = work.tile([128, 1], F32)
            nc.vector.tensor_scalar_add(m1b, m0b, 1.0)  # max_coordinate + 1

            off = work.tile([128, PF], F32)
            nc.vector.tensor_scalar_mul(off, tcls, m1b)  # class_id * (maxc+1)

            # offset coords + areas (kept in SBUF; only the core's first CW
            # slots = its own 10 classes are consumed downstream)
            offc = []
            for qi, t in enumerate((x1, y1, x2, y2)):
                to = work.tile([128, PF], F32, tag=f"coord_o{qi}")
                nc.vector.tensor_tensor(to, t, off, OP.add)
                offc.append(to)
            x1o, y1o, x2o, y2o = offc
            wdt = work.tile([128, PF], F32)
            nc.vector.tensor_tensor(wdt, x2o, x1o, OP.subtract)
            hgt = work.tile([128, PF], F32)
            nc.vector.tensor_tensor(hgt, y2o, y1o, OP.subtract)
            area = work.tile([128, PF], F32)
            nc.vector.tensor_tensor(area, wdt, hgt, OP.mult)

            # ---- stage 2: boxes-output shard decode (1024 original-order rows)
            sd = work.tile([128, NSH // 128, 4], F32)
            nc.scalar.dma_start(out=sd, in_=shd.ap().rearrange("(p a) c -> p a c", p=128))
            sl = work.tile([128, NSH // 128, 2], F32)
            nc.sync.dma_start(out=sl, in_=shl.ap().rearrange("(p a) c -> p a c", p=128))
            sdr = work.tile([128, NSH // 128, 4], F32)
            nc.vector.tensor_scalar_max(sdr, sd, 0.0)
            bx = work.tile([128, NSH // 128, 4], F32)
            nc.vector.scalar_tensor_tensor(bx[:, :, 0], sdr[:, :, 0], -stride, sl[:, :, 0], OP.mult, OP.add)
            nc.vector.scalar_tensor_tensor(bx[:, :, 1], sdr[:, :, 1], -stride, sl[:, :, 1], OP.mult, OP.add)
            nc.vector.scalar_tensor_tensor(bx[:, :, 2], sdr[:, :, 2], stride, sl[:, :, 0], OP.mult, OP.add)
            nc.vector.scalar_tensor_tensor(bx[:, :, 3], sdr[:, :, 3], stride, sl[:, :, 1], OP.mult, OP.add)
            nc.scalar.dma_start(
                out=boxes_out.ap().rearrange("(p a) c -> p a c", p=128), in_=bx
            )

            # ---- stage 3: batched suppression bits for the core's 10 classes
            # The core's own 10 classes occupy slots 0..CW-1 = partitions 0..15
            # of the (128, 85) coord tiles (CW = 16*85).  Per coord plane:
            # extract those slots to one partition (column operand, then
            # partition-broadcast to 128 lanes) and to DRAM (bounce for the
            # free->partition row gather).
            planes = (x1o, y1o, x2o, y2o, area)
            NPP = CW // PF  # 16 partitions holding the core's classes
            colsd = dpool.tile([5, CW], F32)
            colB = [
                work.tile([128, CW], F32, tag=f"colB{qi}", name=f"colB{qi}")
                for qi in range(5)
            ]
            rows = work.tile([128, 5, CPC], F32)
            eng_a = (nc.scalar, nc.sync)
            bc_eng = (None, None, nc.sync, nc.scalar, nc.sync)
            for qi, pt in enumerate(planes):
                eng_a[(qi + 1) % 2].dma_start(
                    out=colsd[qi].rearrange("(b f) -> b f", b=NPP), in_=pt[0:NPP, :]
                )
                if qi < 2:
                    # earliest-needed planes: on-chip broadcast (gpsimd),
                    # running parallel to the queue-limited broadcast-DMAs
                    cols_q = work.tile([1, CW], F32, tag=f"cols{qi}")
                    eng_a[qi % 2].dma_start(
                        out=cols_q[0:1, :].rearrange("a (b f) -> a b f", b=NPP),
                        in_=pt[0:NPP, :],
                    )
                    nc.gpsimd.partition_broadcast(colB[qi], cols_q)
                else:
                    # later planes: broadcast-DMA from the DRAM copy — no
                    # gpsimd write-drain stall for the first vector reader
                    bc_eng[qi].dma_start(
                        out=colB[qi],
                        in_=bass.AP(tensor=colsd.tensor,
                                    offset=colsd.offset + qi * CW,
                                    ap=[[0, 128], [1, CW]]),
                    )
                nc.sync.dma_start(
                    out=rows[:, qi, :],
                    in_=bass.AP(tensor=colsd.tensor,
                                offset=colsd.offset + qi * CW,
                                ap=[[1, 128], [W, CPC]]),
                )

            def rowv(qi):  # (128, CPC, W) j-broadcast view of plane qi's rows
                return _bcast_j(rows[:, qi, :], W)

            # TT max/min run at half DVE rate; tensor_scalar max/min are full
            # rate but the row operand varies per class, so do the four
            # row-dependent ops per class (scalar = this class's row coord)
            # and fuse min+subtract into one scalar_tensor_tensor.
            # plane-major order: the first reader of each freshly-broadcast
            # colB plane stalls on the gpsimd drain, so pay that stall once
            # per plane (not once per class-chain link)
            sh3 = [128, CPC, W]
            ltx = work.tile(sh3, F32)
            lty = work.tile(sh3, F32)
            wq = work.tile(sh3, F32)
            hq = work.tile(sh3, F32)
            for c in range(CPC):
                nc.vector.tensor_scalar_max(
                    ltx[:, c, :], colB[0][:, c * W : (c + 1) * W], rows[:, 0, c : c + 1]
                )
            for c in range(CPC):
                nc.vector.tensor_scalar_max(
                    lty[:, c, :], colB[1][:, c * W : (c + 1) * W], rows[:, 1, c : c + 1]
                )
            for c in range(CPC):
                nc.vector.scalar_tensor_tensor(
                    wq[:, c, :], colB[2][:, c * W : (c + 1) * W], rows[:, 2, c : c + 1],
                    ltx[:, c, :], OP.min, OP.subtract,
                )
            for c in range(CPC):
                nc.vector.scalar_tensor_tensor(
                    hq[:, c, :], colB[3][:, c * W : (c + 1) * W], rows[:, 3, c : c + 1],
                    lty[:, c, :], OP.min, OP.subtract,
                )
            hc = work.tile(sh3, F32)
            nc.scalar.activation(hc, hq, mybir.ActivationFunctionType.Relu)
            inter = work.tile(sh3, F32)
            nc.vector.scalar_tensor_tensor(inter, wq, 0.0, hc, OP.max, OP.mult)
            asum = work.tile(sh3, F32)
            nc.vector.tensor_tensor(
                asum, colB[4].rearrange("p (c j) -> p c j", c=CPC), rowv(4), OP.add
            )

            # suppression bits in bf16 (0/1 exact): halves the collapse DMA
            # bytes; is_gt split per class so each class's collapse starts
            # as soon as its bits exist instead of after the whole batch
            BF16 = mybir.dt.bfloat16
            sbits = work.tile(sh3, BF16)
            keep_a = work.tile([CPC, W], BF16)
            keep_b = work.tile([CPC, W], BF16)
            nc.vector.memset(keep_a, 1.0)
            nc.vector.memset(keep_b, 1.0)
            s3 = work.tile(sh3, F32)
            nc.scalar.mul(s3, asum, 1.0 / 3.0)
            nc.vector.tensor_tensor(sbits, inter, s3, OP.is_gt)
            # partition flip via DRAM bounce: contiguous (k, c, j) write, then
            # strided read back as (c-partition, k-major).  Split in row
            # quarters so sweep steps 0-31 only wait for the first quarter
            # and the rest transfers underneath the running sweep.
            KQ = 4
            sshalf = [
                work.tile([CPC, KQ * W], BF16, tag=f"ssweep{h}", name=f"ssweep{h}")
                for h in range(128 // KQ)
            ]
            for h in range(128 // KQ):
                sb_d = dpool.tile([KQ * CW], BF16, tag=f"sb_d{h}")
                nc.scalar.dma_start(
                    out=sb_d.rearrange("(k f) -> k f", k=KQ),
                    in_=sbits[KQ * h : KQ * (h + 1)],
                )
                rd_eng = nc.sync if h % 2 == 0 else nc.scalar
                rd_eng.dma_start(
                    out=sshalf[h].rearrange("c (k j) -> c k j", k=KQ),
                    in_=bass.AP(tensor=sb_d.tensor, offset=sb_d.offset,
                                ap=[[W, CPC], [CW, KQ], [1, W]]),
                )

            # ---- stage 4: greedy suppression sweep, all classes in parallel.
            # Ping-pong buffers instead of in-place so each step is a clean
            # cross-tile RAW; keep[] only ever decreases, so column j's final
            # value (written at step j-1) is the elementwise min of the two.
            bufs = (keep_a, keep_b)
            for k in range(nsweep):
                src, dst = bufs[k % 2], bufs[(k + 1) % 2]
                stile = sshalf[k // KQ]
                kk = k % KQ
                # dst[j] = (S[k,j]*src[k]) < src[j]   for j in (k, smax)
                nc.vector.scalar_tensor_tensor(
                    dst[:, k + 1 : smax],
                    stile[:, kk * W + k + 1 : kk * W + smax],
                    src[:, k : k + 1],
                    src[:, k + 1 : smax],
                    OP.mult,
                    OP.is_lt,
                )

            keepm = work.tile([CPC, W], BF16)
            nc.vector.tensor_tensor(keepm, keep_a, keep_b, OP.min)
            keepf = work.tile([CPC, W], F32)
            nc.vector.tensor_copy(keepf, keepm)
            nc.sync.dma_start(out=keep_out.ap(), in_=keepf)

    nc.compile()
    return nc


def kernel(deltas, locations, scores, class_ids, stride):
    global LAST_RUN
    deltas = np.ascontiguousarray(np.asarray(deltas, dtype=np.float32))
    locations = np.ascontiguousarray(np.asarray(locations, dtype=np.float32))
    scores = np.asarray(scores, dtype=np.float32)
    class_ids = np.asarray(class_ids, dtype=np.int32)
    s = float(stride)
    assert deltas.shape == (N, 4) and locations.shape == (N, 2)

    # ---- host: ordering + class grouping (layout only; all math on device)
    order = np.argsort(-scores, kind="stable").astype(np.int32)
    rank = np.empty(N, np.int64)
    rank[order] = np.arange(N)
    sort_key = np.lexsort((rank, class_ids))          # class-major, score-rank minor
    counts = np.bincount(class_ids, minlength=NCLS)
    smax = int(counts.max())
    assert smax <= 129, f"class size {smax} exceeds supported 129"

    starts = np.zeros(NCLS, np.int64)
    starts[1:] = np.cumsum(counts)[:-1]
    sorted_cls = class_ids[sort_key].astype(np.int64)
    within = np.arange(N) - np.repeat(starts, counts)
    slots = sorted_cls * W + within                    # padded slot per box

    pd_c = np.zeros((4, NCLS, W), np.float32)
    pl_c = np.zeros((2, NCLS, W), np.float32)
    pcls_c = np.zeros((NCLS, W), np.float32)
    pd_c.reshape(4, FLAT)[:, slots] = deltas[sort_key].T
    pl_c.reshape(2, FLAT)[:, slots] = locations[sort_key].T
    pcls_c.reshape(FLAT)[slots] = sorted_cls.astype(np.float32)

    key = (smax, s)
    if key not in _PROGRAM_CACHE:
        _PROGRAM_CACHE[key] = _build_program(smax, s)
    nc = _PROGRAM_CACHE[key]

    in_maps = []
    for k in range(NCORES):
        r = -(k * CPC)
        in_maps.append(
            {
                "pd": np.ascontiguousarray(np.roll(pd_c, r, axis=1).reshape(4, FLAT)),
                "pl": np.ascontiguousarray(np.roll(pl_c, r, axis=1).reshape(2, FLAT)),
                "pcls": np.ascontiguousarray(np.roll(pcls_c, r, axis=0).reshape(FLAT)),
                "shd": deltas[k * NSH : (k + 1) * NSH],
                "shl": locations[k * NSH : (k + 1) * NSH],
            }
        )

    res = run_bass_kernel_spmd(nc, in_maps, list(range(NCORES)))
    LAST_RUN = res
    results = res.results

    boxes = np.concatenate([results[k]["boxes_out"] for k in range(NCORES)], axis=0)
    keep_pad = np.concatenate(
        [results[k]["keep_out"].reshape(CPC, W) for k in range(NCORES)], axis=0
    )
    keep_orig = np.empty(N, bool)
    keep_orig[sort_key] = keep_pad.reshape(FLAT)[slots] > 0.5
    return boxes, keep_orig, order
